# revision 9
# baseline (speedup 1.0000x reference)
"""Trainium2 Bass kernel for nn_BinaryMemory (retrieval_knn).

reference:
    gated = sigmoid(query @ W.T + b)                      # [1, D], D=4096
    sims  = 1 - mean(|memory - gated|, axis=-1)           # [N],   N=16384
    mask  = sims >= 0.8

Sharding (8 cores, no collectives): shard the D axis. Core c owns
d-chunk [c*512, (c+1)*512):
  - W rows c*512..c*512+511  -> computes gated[c*512:(c+1)*512] locally
    (dot products via scalar_tensor_tensor with sum-accumulate on DVE)
  - memory[:, c*512:(c+1)*512] -> partial L1 sums over its d-chunk for
    all 16384 rows
  - outputs partial sums [128, 128]; host reindexes, sums the 8 cores'
    partials and applies sims = 1 - s/D, mask = sims >= 0.8.

The |m-g| + window-sum work is split across three engine routes so no
single engine is the tail:
  A: DVE subtract + DVE tensor_reduce(abs, X) [128,8,512]->[128,8]
  B: DVE subtract + 8x ScalarE Abs-with-accumulate
  C: GpSimd subtract + 8x ScalarE Abs-with-accumulate

Memory tile t holds rows t*1024..t*1024+1023; partition p holds the 8
consecutive rows t*1024+p*8+j (16 KB contiguous DMA runs). Per-core HBM
traffic ~40 MB (memory regime, ~320 GB/s achievable => ~125 us floor).
"""
import sys

sys.path.insert(0, "/opt/trn_rl_repo")

import numpy as np

import concourse.bacc as bacc
import concourse.mybir as mybir
import concourse.tile as tile
from concourse.bass_utils import run_bass_kernel_spmd

N_CORES = 8
D = 4096
N = 16384
D_SH = D // N_CORES          # 512 dims per core
W_TILES = D_SH // 128        # 4 gate-weight tiles [128, 4096]
GP = 8                       # row-groups packed per memory tile
M_TILES = N // (128 * GP)    # 16 memory tiles [128, 8*512]
THRESHOLD = 0.8

# engine route per memory tile: C early (GpSimd free), A late-ish (DVE
# busy with the gate early), B fills the rest
ROUTES = "CBCBCABCBACBBABB"
assert len(ROUTES) == M_TILES

_CACHE = {}


def _build():
    f32 = mybir.dt.float32
    nc = bacc.Bacc(
        "TRN2", target_bir_lowering=False, debug=False, num_devices=N_CORES
    )

    query = nc.dram_tensor("query", [1, D], f32, kind="ExternalInput")
    w = nc.dram_tensor("w", [D_SH, D], f32, kind="ExternalInput")
    b = nc.dram_tensor("b", [D_SH], f32, kind="ExternalInput")
    mem = nc.dram_tensor("mem", [N, D_SH], f32, kind="ExternalInput")
    ident = nc.dram_tensor("ident", [128, 128], f32, kind="ExternalInput")
    partials = nc.dram_tensor(
        "partials", [128, M_TILES * GP], f32, kind="ExternalOutput"
    )

    with tile.TileContext(nc) as tc:
        with (
            tc.tile_pool(name="const", bufs=1) as cpool,
            tc.tile_pool(name="big", bufs=7) as bpool,
            tc.tile_pool(name="diff", bufs=3) as dpool,
            tc.tile_pool(name="absout", bufs=2) as apool,
            tc.tile_pool(name="small", bufs=1) as spool,
            tc.tile_pool(name="psum", bufs=1, space="PSUM") as ppool,
            tc.tile_pool(name="dram", bufs=1, space="DRAM") as drpool,
        ):
            id_sb = cpool.tile([128, 128], f32, tag="ident")
            nc.sync.dma_start(out=id_sb[:], in_=ident[:])

            # ---- gate: z[j] = sum_d W[j, d] * q[d], j = wt*128 + p ----
            # scalar-engine HWDGE ring is otherwise idle -> low-latency
            q_b = dpool.tile([128, D], f32, tag="diff")
            nc.scalar.dma_start(out=q_b[:], in_=query[:].to_broadcast((128, D)))

            z_col = spool.tile([128, W_TILES], f32, tag="zcol")
            for wt in range(W_TILES):
                w_tile = bpool.tile([128, D], f32, tag="m")
                w_eng = nc.sync if wt % 2 == 0 else nc.gpsimd
                w_eng.dma_start(
                    out=w_tile[:], in_=w[wt * 128 : (wt + 1) * 128, :]
                )
                scratch = dpool.tile([128, D], f32, tag="diff")
                nc.vector.scalar_tensor_tensor(
                    out=scratch[:],
                    in0=w_tile[:],
                    scalar=1.0,
                    in1=q_b[:],
                    op0=mybir.AluOpType.mult,
                    op1=mybir.AluOpType.mult,
                    accum_out=z_col[:, wt : wt + 1],
                )

            # z + b, sigmoid (column layout: [p, wt] = j = wt*128+p)
            b_col = spool.tile([128, W_TILES], f32, tag="bcol")
            nc.scalar.dma_start(
                out=b_col[:], in_=b[:].rearrange("(t p) -> p t", p=128)
            )
            g_col = spool.tile([128, W_TILES], f32, tag="gcol")
            nc.vector.tensor_add(g_col[:], z_col[:], b_col[:])
            nc.scalar.activation(
                g_col[:], g_col[:], mybir.ActivationFunctionType.Sigmoid
            )

            # transpose to row layout; bounce through DRAM to broadcast
            g_ps = ppool.tile([W_TILES, 128], f32, tag="gps")
            nc.tensor.transpose(g_ps[:], g_col[:], id_sb[:])
            g_row = spool.tile([W_TILES, 128], f32, tag="grow")
            nc.vector.tensor_copy(g_row[:], g_ps[:])
            g_dram = drpool.tile([D_SH], f32, tag="gdram")
            nc.scalar.dma_start(
                out=g_dram[:].rearrange("(t p) -> t p", p=128), in_=g_row[:]
            )
            # g_rep[p, (j d)] = gated[d] replicated for all partitions/groups
            g_rep = cpool.tile([128, GP * D_SH], f32, tag="grep")
            nc.scalar.dma_start(
                out=g_rep[:].rearrange("p (j d) -> p j d", j=GP),
                in_=g_dram[:].unsqueeze(0).unsqueeze(0).to_broadcast((128, GP, D_SH)),
            )

            # ---- sims partials ----
            # tile t: partition p, free (j, d) = mem[t*1024 + p*8 + j, d]
            memv = mem[:].rearrange("(t p j) d -> t p j d", p=128, j=GP)
            sums = spool.tile([128, M_TILES * GP], f32, tag="sums")
            for t in range(M_TILES):
                route = ROUTES[t]
                m_tile = bpool.tile([128, GP * D_SH], f32, tag="m")
                dma_eng = nc.sync if t % 2 == 0 else nc.gpsimd
                dma_eng.dma_start(
                    out=m_tile[:].rearrange("p (j d) -> p j d", j=GP),
                    in_=memv[t],
                )
                diff = dpool.tile([128, GP * D_SH], f32, tag="diff")
                sub_eng = nc.gpsimd if route == "C" else nc.vector
                sub_eng.tensor_sub(diff[:], m_tile[:], g_rep[:])
                if route == "A":
                    nc.vector.tensor_reduce(
                        out=sums[:, t * GP : (t + 1) * GP],
                        in_=diff[:].rearrange("p (j d) -> p j d", j=GP),
                        axis=mybir.AxisListType.X,
                        op=mybir.AluOpType.add,
                        apply_absolute_value=True,
                    )
                else:
                    for j in range(GP):
                        a_out = apool.tile([128, D_SH], f32, tag="absout")
                        col = t * GP + j
                        nc.scalar.activation(
                            a_out[:],
                            diff[:, j * D_SH : (j + 1) * D_SH],
                            mybir.ActivationFunctionType.Abs,
                            accum_out=sums[:, col : col + 1],
                        )

            nc.sync.dma_start(out=partials[:], in_=sums[:])

    nc.compile()
    return nc


def _get_nc():
    if "nc" not in _CACHE:
        _CACHE["nc"] = _build()
    return _CACHE["nc"]


def kernel(query, W, b, memory, _trace=False, _return_raw=False):
    query = np.ascontiguousarray(np.asarray(query, dtype=np.float32))
    W = np.asarray(W, dtype=np.float32)
    b = np.asarray(b, dtype=np.float32)
    memory = np.asarray(memory, dtype=np.float32)
    ident = np.eye(128, dtype=np.float32)

    in_maps = []
    for c in range(N_CORES):
        sl = slice(c * D_SH, (c + 1) * D_SH)
        in_maps.append(
            {
                "query": query,
                "w": np.ascontiguousarray(W[sl, :]),
                "b": np.ascontiguousarray(b[sl]),
                "mem": np.ascontiguousarray(memory[:, sl]),
                "ident": ident,
            }
        )

    nc = _get_nc()
    res = run_bass_kernel_spmd(
        nc, in_maps, list(range(N_CORES)), trace=_trace
    )

    total = np.zeros(N, dtype=np.float64)
    for c in range(N_CORES):
        mat = res.results[c]["partials"]  # [128 (p), 128 (t*8+j)]
        # row n = t*1024 + p*8 + j
        part = mat.reshape(128, M_TILES, GP).transpose(1, 0, 2).reshape(N)
        total += part.astype(np.float64)
    sims = (1.0 - total / D).astype(np.float32)
    mask = sims >= THRESHOLD
    if _return_raw:
        return (sims, mask), res
    return sims, mask


# revision 12
# speedup vs baseline: 1.0852x; 1.0852x over previous
"""Trainium2 Bass kernel for nn_BinaryMemory (retrieval_knn).

reference:
    gated = sigmoid(query @ W.T + b)                      # [1, D], D=4096
    sims  = 1 - mean(|memory - gated|, axis=-1)           # [N],   N=16384
    mask  = sims >= 0.8

Sharding (8 cores, no collectives): shard the D axis. Core c owns
d-chunk [c*512, (c+1)*512):
  - W rows c*512..c*512+511  -> computes gated[c*512:(c+1)*512] locally
    (dot products via scalar_tensor_tensor with sum-accumulate on DVE)
  - memory[:, c*512:(c+1)*512] -> partial L1 sums over its d-chunk for
    all 16384 rows
  - outputs partial sums [128, 128]; host reindexes, sums the 8 cores'
    partials and applies sims = 1 - s/D, mask = sims >= 0.8.

The |m-g| + window-sum work is split across three engine routes so no
single engine is the tail:
  A: DVE subtract + DVE tensor_reduce(abs, X) [128,8,512]->[128,8]
  B: DVE subtract + 8x ScalarE Abs-with-accumulate
  C: GpSimd subtract + 8x ScalarE Abs-with-accumulate

Memory tile t holds rows t*1024..t*1024+1023; partition p holds the 8
consecutive rows t*1024+p*8+j (16 KB contiguous DMA runs). Per-core HBM
traffic ~40 MB (memory regime, ~320 GB/s achievable => ~125 us floor).
"""
import sys

sys.path.insert(0, "/opt/trn_rl_repo")

import numpy as np

import concourse.bacc as bacc
import concourse.mybir as mybir
import concourse.tile as tile
from concourse.bass_utils import run_bass_kernel_spmd

N_CORES = 8
D = 4096
N = 16384
D_SH = D // N_CORES          # 512 dims per core
W_TILES = D_SH // 128        # 4 gate-weight tiles [128, 4096]
GP = 8                       # row-groups packed per memory tile
M_TILES = N // (128 * GP)    # 16 memory tiles [128, 8*512]
THRESHOLD = 0.8

# engine route per memory tile. GpSimd tensor ops contend with DVE for
# SBUF ports (measured: both slow ~1.5x when concurrent), so no C route.
# A (DVE-only abs-reduce) placed at the tail for a shorter drain.
ROUTES = "BBBBBBBBBBBBBABA"
assert len(ROUTES) == M_TILES

_CACHE = {}


def _build():
    f32 = mybir.dt.float32
    nc = bacc.Bacc(
        "TRN2", target_bir_lowering=False, debug=False, num_devices=N_CORES
    )

    query = nc.dram_tensor("query", [1, D], f32, kind="ExternalInput")
    w = nc.dram_tensor("w", [D_SH, D], f32, kind="ExternalInput")
    b = nc.dram_tensor("b", [D_SH], f32, kind="ExternalInput")
    mem = nc.dram_tensor("mem", [N, D_SH], f32, kind="ExternalInput")
    ident = nc.dram_tensor("ident", [128, 128], f32, kind="ExternalInput")
    partials = nc.dram_tensor(
        "partials", [128, M_TILES * GP], f32, kind="ExternalOutput"
    )

    with tile.TileContext(nc) as tc:
        with (
            tc.tile_pool(name="const", bufs=1) as cpool,
            tc.tile_pool(name="big", bufs=6) as bpool,
            tc.tile_pool(name="diff", bufs=4) as dpool,
            tc.tile_pool(name="absout", bufs=2) as apool,
            tc.tile_pool(name="small", bufs=1) as spool,
            tc.tile_pool(name="psum", bufs=1, space="PSUM") as ppool,
            tc.tile_pool(name="dram", bufs=1, space="DRAM") as drpool,
        ):
            id_sb = cpool.tile([128, 128], f32, tag="ident")
            nc.sync.dma_start(out=id_sb[:], in_=ident[:])

            # ---- gate: z[j] = sum_d W[j, d] * q[d], j = wt*128 + p ----
            # scalar-engine HWDGE ring is otherwise idle -> low-latency
            q_b = dpool.tile([128, D], f32, tag="diff")
            nc.scalar.dma_start(out=q_b[:], in_=query[:].to_broadcast((128, D)))

            z_col = spool.tile([128, W_TILES], f32, tag="zcol")
            for wt in range(W_TILES):
                w_tile = bpool.tile([128, D], f32, tag="m")
                w_eng = nc.sync if wt % 2 == 0 else nc.gpsimd
                w_eng.dma_start(
                    out=w_tile[:], in_=w[wt * 128 : (wt + 1) * 128, :]
                )
                scratch = dpool.tile([128, D], f32, tag="diff")
                nc.vector.scalar_tensor_tensor(
                    out=scratch[:],
                    in0=w_tile[:],
                    scalar=1.0,
                    in1=q_b[:],
                    op0=mybir.AluOpType.mult,
                    op1=mybir.AluOpType.mult,
                    accum_out=z_col[:, wt : wt + 1],
                )

            # z + b, sigmoid (column layout: [p, wt] = j = wt*128+p)
            b_col = spool.tile([128, W_TILES], f32, tag="bcol")
            nc.scalar.dma_start(
                out=b_col[:], in_=b[:].rearrange("(t p) -> p t", p=128)
            )
            g_col = spool.tile([128, W_TILES], f32, tag="gcol")
            nc.vector.tensor_add(g_col[:], z_col[:], b_col[:])
            nc.scalar.activation(
                g_col[:], g_col[:], mybir.ActivationFunctionType.Sigmoid
            )

            # transpose to row layout; bounce through DRAM to broadcast
            g_ps = ppool.tile([W_TILES, 128], f32, tag="gps")
            nc.tensor.transpose(g_ps[:], g_col[:], id_sb[:])
            g_row = spool.tile([W_TILES, 128], f32, tag="grow")
            nc.vector.tensor_copy(g_row[:], g_ps[:])
            g_dram = drpool.tile([D_SH], f32, tag="gdram")
            nc.scalar.dma_start(
                out=g_dram[:].rearrange("(t p) -> t p", p=128), in_=g_row[:]
            )
            # g_rep[p, (j d)] = gated[d] replicated for all partitions/groups
            g_rep = cpool.tile([128, GP * D_SH], f32, tag="grep")
            nc.scalar.dma_start(
                out=g_rep[:].rearrange("p (j d) -> p j d", j=GP),
                in_=g_dram[:].unsqueeze(0).unsqueeze(0).to_broadcast((128, GP, D_SH)),
            )

            # ---- sims partials ----
            # tile t: partition p, free (j, d) = mem[t*1024 + p*8 + j, d]
            memv = mem[:].rearrange("(t p j) d -> t p j d", p=128, j=GP)
            sums = spool.tile([128, M_TILES * GP], f32, tag="sums")
            for t in range(M_TILES):
                route = ROUTES[t]
                m_tile = bpool.tile([128, GP * D_SH], f32, tag="m")
                dma_eng = nc.sync if t % 2 == 0 else nc.gpsimd
                dma_eng.dma_start(
                    out=m_tile[:].rearrange("p (j d) -> p j d", j=GP),
                    in_=memv[t],
                )
                diff = dpool.tile([128, GP * D_SH], f32, tag="diff")
                nc.vector.tensor_sub(diff[:], m_tile[:], g_rep[:])
                if route == "A":
                    nc.vector.tensor_reduce(
                        out=sums[:, t * GP : (t + 1) * GP],
                        in_=diff[:].rearrange("p (j d) -> p j d", j=GP),
                        axis=mybir.AxisListType.X,
                        op=mybir.AluOpType.add,
                        apply_absolute_value=True,
                    )
                else:
                    for j in range(GP):
                        a_out = apool.tile([128, D_SH], f32, tag="absout")
                        col = t * GP + j
                        nc.scalar.activation(
                            a_out[:],
                            diff[:, j * D_SH : (j + 1) * D_SH],
                            mybir.ActivationFunctionType.Abs,
                            accum_out=sums[:, col : col + 1],
                        )

            nc.sync.dma_start(out=partials[:], in_=sums[:])

    nc.compile()
    return nc


def _get_nc():
    if "nc" not in _CACHE:
        _CACHE["nc"] = _build()
    return _CACHE["nc"]


def kernel(query, W, b, memory, _trace=False, _return_raw=False):
    query = np.ascontiguousarray(np.asarray(query, dtype=np.float32))
    W = np.asarray(W, dtype=np.float32)
    b = np.asarray(b, dtype=np.float32)
    memory = np.asarray(memory, dtype=np.float32)
    ident = np.eye(128, dtype=np.float32)

    in_maps = []
    for c in range(N_CORES):
        sl = slice(c * D_SH, (c + 1) * D_SH)
        in_maps.append(
            {
                "query": query,
                "w": np.ascontiguousarray(W[sl, :]),
                "b": np.ascontiguousarray(b[sl]),
                "mem": np.ascontiguousarray(memory[:, sl]),
                "ident": ident,
            }
        )

    nc = _get_nc()
    res = run_bass_kernel_spmd(
        nc, in_maps, list(range(N_CORES)), trace=_trace
    )

    total = np.zeros(N, dtype=np.float64)
    for c in range(N_CORES):
        mat = res.results[c]["partials"]  # [128 (p), 128 (t*8+j)]
        # row n = t*1024 + p*8 + j
        part = mat.reshape(128, M_TILES, GP).transpose(1, 0, 2).reshape(N)
        total += part.astype(np.float64)
    sims = (1.0 - total / D).astype(np.float32)
    mask = sims >= THRESHOLD
    if _return_raw:
        return (sims, mask), res
    return sims, mask
